# revision 8
# baseline (speedup 1.0000x reference)
"""Contour -> distance map kernel for 8 Trainium2 NeuronCores.

Math. Per polygon p (one per core), the output map is
  dmap = wind(i,j) * dist(i,j) / global_max,
with wind the (tanh-smoothed, here exact-integer) winding number and dist
the distance from pixel (i/256, j/256) to the nearest contour VERTEX.

Both factors are piecewise structured along each pixel row i:

* Winding: cast a ray in +j. Edge crossings with the line u=i/256 happen at
  row-only-dependent positions v*_k(i); wind(i,j) = |sum of signed crossings
  right of j| = suffix-sum of a per-row signed histogram h(i,q) of the
  pixel-quantized crossing positions (exact: bins align with the j grid).

* Distance: dist^2(i,j) = min_k (j/256 - vy_k)^2 + a_k(i) is the lower
  envelope of 64 equal-curvature parabolas in v=j/256. Felzenszwalb's
  envelope (host, O(K) per row) makes it piecewise quadratic:
  dist^2 = C(i,j) - (j/256)*B2(i,j) with B2 = 2*vy_owner and
  C = vy_owner^2 + a_owner + (j/256)^2 piecewise-constant in j -> both are
  prefix sums of per-row jump histograms gB2, gC (the v^2 term folds into
  gC's bins as finite differences).

Device work per core: DMA 3 histograms (128c x 512) in, 12 accumulating
matmuls against constant triangular 0/1 matrices render the three maps
W (winding, signed), B2, D=C into PSUM, then 4 DVE tensor-tensor ops:
  W2 = W*W; m2 = B2*vj; d2 = D - m2; pr = W2*d2   [pr = (wind*dist)^2]
Host epilogue: sqrt + global max normalize (same role as the original
baseline's host epilogue).

Host prep is O(rows*K) per polygon (crossings + envelope), analogous to the
previous baseline's phi/W-matrix prep; all per-pixel work stays on device.
"""

import numpy as np

SIZE = 256
NCORES = 8
K = 64
DMA_SPLIT = 4
INF = 1e30

_CACHE = {}


def _build_program(reps=1, skip=()):
    import concourse.bacc as bacc
    import concourse.tile as tile
    from concourse import mybir

    f32 = mybir.dt.float32
    ALU = mybir.AluOpType
    AF = mybir.ActivationFunctionType

    nc = bacc.Bacc("TRN2", target_bir_lowering=False, debug=False,
                   num_devices=NCORES)

    hw_d = nc.dram_tensor("hw", [128, 512], f32, kind="ExternalInput")
    gb_d = nc.dram_tensor("gb", [128, 512], f32, kind="ExternalInput")
    gc_d = nc.dram_tensor("gc", [128, 512], f32, kind="ExternalInput")
    uw_d = nc.dram_tensor("uw", [128, 2, 256], f32, kind="ExternalInput")
    up_d = nc.dram_tensor("up", [128, 2, 256], f32, kind="ExternalInput")
    v1_d = nc.dram_tensor("v1", [128, 512], f32, kind="ExternalInput")
    pr_d = nc.dram_tensor("pr", [128, 512], f32, kind="ExternalOutput")

    with tile.TileContext(nc) as tc:
        with (
            tc.tile_pool(name="const", bufs=1) as const,
            tc.tile_pool(name="hbuf", bufs=2) as hbuf,
            tc.tile_pool(name="psum", bufs=2, space="PSUM") as psum_pool,
            tc.tile_pool(name="work", bufs=2) as work,
        ):
            uw_s = const.tile([128, 2, 256], f32)
            up_s = const.tile([128, 2, 256], f32)
            v1_s = const.tile([128, 512], f32)
            nc.sync.dma_start(uw_s[:], uw_d[:])
            nc.sync.dma_start(up_s[:], up_d[:])
            nc.sync.dma_start(v1_s[:], v1_d[:])

            CW = 512 // DMA_SPLIT

            def body():
                hw_s = hbuf.tile([128, 512], f32, tag="hw")
                gb_s = hbuf.tile([128, 512], f32, tag="gb")
                gc_s = hbuf.tile([128, 512], f32, tag="gc")
                for s, d in ((hw_s, hw_d), (gb_s, gb_d), (gc_s, gc_d)):
                    for ch in range(DMA_SPLIT):
                        sl = slice(ch * CW, (ch + 1) * CW)
                        nc.sync.dma_start(s[:, sl], d[:, sl])

                W_t = psum_pool.tile([128, 512], f32, tag="W")
                B_t = psum_pool.tile([128, 512], f32, tag="B")
                D_t = psum_pool.tile([128, 512], f32, tag="D")
                # winding first so its DVE square overlaps the B/D matmuls
                for src, dst, rhs in ((hw_s, W_t, uw_s), (gb_s, B_t, up_s),
                                      (gc_s, D_t, up_s)):
                    for bi in range(2):
                        for bq in range(2):
                            col = (bq * 2 + bi) * 128
                            nc.tensor.matmul(
                                dst[:, 256 * bi:256 * bi + 256],
                                src[:, col:col + 128],
                                rhs[:, bq, :],
                                start=(bq == 0), stop=(bq == 1))

                W2 = work.tile([128, 512], f32, tag="W2")
                m2 = work.tile([128, 512], f32, tag="m2")
                d2 = work.tile([128, 512], f32, tag="d2")
                pr = work.tile([128, 512], f32, tag="pr")
                nc.scalar.activation(W2[:], W_t[:], AF.Square)
                nc.vector.tensor_tensor(m2[:], B_t[:], v1_s[:], op=ALU.mult)
                nc.vector.tensor_tensor(d2[:], D_t[:], m2[:], op=ALU.subtract)
                nc.vector.tensor_tensor(pr[:], W2[:], d2[:], op=ALU.mult)
                for ch in range(DMA_SPLIT):
                    sl = slice(ch * CW, (ch + 1) * CW)
                    nc.sync.dma_start(pr_d[:, sl], pr[:, sl])

            if reps > 1:
                with tc.For_i(0, reps, 1,
                              hint_engines=(mybir.EngineType.PE,
                                            mybir.EngineType.DVE)):
                    body()
            else:
                body()

    nc.compile()
    return nc


def _winding_hist(V):
    """Signed histogram of pixel-quantized ray crossings, (256 i, 256 q).
    wind(i, j) = |sum_{q-bin >= j} h[i, q]|."""
    A, B = V, np.roll(V, -1, axis=0)
    ui = (np.arange(SIZE) / SIZE)[:, None]
    Au, Av, Bu, Bv = A[:, 0][None], A[:, 1][None], B[:, 0][None], B[:, 1][None]
    du = Bu - Au
    active = ((Au <= ui) & (ui < Bu)) | ((Bu <= ui) & (ui < Au))
    with np.errstate(divide="ignore", invalid="ignore"):
        vstar = Av + (ui - Au) / du * (Bv - Av)
    s = np.where(du > 0, 1.0, -1.0)
    q = np.where(active, np.clip(np.ceil(SIZE * vstar), 0, SIZE), 0)
    q = q.astype(np.int64)
    sgn = np.where(active, s, 0.0)
    h = np.zeros((SIZE, SIZE))
    ii = np.broadcast_to(np.arange(SIZE)[:, None], q.shape)
    m = q > 0
    np.add.at(h, (ii[m], q[m] - 1), sgn[m])
    return h


def _envelope_hists(V):
    """Vectorized Felzenszwalb lower envelope over the 256 rows.
    Returns jump histograms gB2 (2*vy_owner) and gC (vy^2+a owner, with the
    v_j^2 term folded in); maps are prefix sums over q <= j."""
    R = SIZE
    order = np.argsort(V[:, 1], kind="stable")
    p = V[order, 1]
    u = (np.arange(R) / SIZE)[:, None]
    a = (u - V[order, 0][None]) ** 2
    c = a + p[None] ** 2
    stk_k = np.zeros((R, K), np.int64)
    stk_z = np.full((R, K), -INF)
    top = np.zeros(R, np.int64)
    rows = np.arange(R)

    def isect(q):
        kt = stk_k[rows, top]
        dp = p[q] - p[kt]
        ct = c[rows, kt]
        with np.errstate(divide="ignore", invalid="ignore"):
            z = (c[:, q] - ct) / (2 * dp)
        return np.where(dp == 0, np.where(c[:, q] >= ct, INF, -INF), z)

    for q in range(1, K):
        while True:
            z = isect(q)
            pop = (z <= stk_z[rows, top]) & (top > 0)
            if not pop.any():
                break
            top = top - pop
        z = isect(q)
        skip = z >= INF / 2
        repl = (~skip) & (z <= stk_z[rows, top])  # dominate-all: new base
        push = (~skip) & (~repl)
        stk_k[rows[repl], top[repl]] = q
        top = top + push
        stk_k[rows[push], top[push]] = q
        stk_z[rows[push], top[push]] = z[push]

    gB2 = np.zeros((R, SIZE))
    gC = np.zeros((R, SIZE))
    prevB = np.zeros(R)
    prevC = np.zeros(R)
    for s in range(int(top.max()) + 1):
        valid = top >= s
        ss = np.minimum(s, top)
        k_s = stk_k[rows, ss]
        z_s = stk_z[rows, ss]
        j0 = np.clip(np.ceil(SIZE * z_s), 0, 256).astype(np.int64)
        vB = 2 * p[k_s]
        vC = c[rows, k_s]
        m = valid & (j0 < 256)
        np.add.at(gB2, (rows[m], j0[m]), (vB - prevB)[m])
        np.add.at(gC, (rows[m], j0[m]), (vC - prevC)[m])
        prevB = np.where(m, vB, prevB)
        prevC = np.where(m, vC, prevC)
    vq = np.arange(SIZE) / SIZE
    gC = gC + np.concatenate([[0.0], np.diff(vq ** 2)])[None, :]
    return gB2, gC


def _lhsT(g):
    """(256 i, 256 q) map histogram -> matmul lhsT (128 c, 512) with column
    order (bq, bi, m): lhsT[c, bq*256+bi*128+m] = g[bi*128+m, bq*128+c]."""
    g4 = np.asarray(g, np.float32).reshape(2, 128, 2, 128)  # [bi, m, bq, c]
    return np.ascontiguousarray(g4.transpose(3, 2, 0, 1).reshape(128, 512))


def _consts():
    if "consts" in _CACHE:
        return _CACHE["consts"]
    b = np.arange(128)[:, None, None] + 128 * np.arange(2)[None, :, None]
    j = np.arange(256)[None, None, :]
    uw = (j <= b).astype(np.float32)                  # wind: suffix incl.
    up = (b <= j).astype(np.float32)                  # maps: prefix incl.
    v1 = np.ascontiguousarray(
        np.broadcast_to((np.arange(256) / SIZE).astype(np.float32),
                        (128, 2, 256)).reshape(128, 512))
    _CACHE["consts"] = {"uw": uw, "up": up, "v1": v1}
    return _CACHE["consts"]


def _host_inputs(contour):
    C = contour.reshape(NCORES, K, 2).astype(np.float64)
    consts = _consts()
    maps = []
    for pidx in range(NCORES):
        V = C[pidx]
        h = _winding_hist(V)
        gB2, gC = _envelope_hists(V)
        maps.append({"hw": _lhsT(h), "gb": _lhsT(gB2), "gc": _lhsT(gC),
                     **consts})
    return maps


def _get_executor(reps=1, skip=()):
    """Build (once) a reusable jitted SPMD executor over the 8 cores."""
    key = ("exec", reps, tuple(sorted(skip)))
    if key in _CACHE:
        return _CACHE[key]

    import jax
    from jax.sharding import Mesh, PartitionSpec, NamedSharding
    from jax.experimental.shard_map import shard_map
    import concourse.mybir as mybir
    from concourse.bass2jax import _bass_exec_p, install_neuronx_cc_hook

    install_neuronx_cc_hook()
    nckey = ("nc", reps, tuple(sorted(skip)))
    if nckey not in _CACHE:
        _CACHE[nckey] = _build_program(reps=reps, skip=skip)
    nc = _CACHE[nckey]
    partition_name = (nc.partition_id_tensor.name
                      if nc.partition_id_tensor else None)

    in_names, out_names, out_avals, zero_outs = [], [], [], []
    for alloc in nc.m.functions[0].allocations:
        if not isinstance(alloc, mybir.MemoryLocationSet):
            continue
        name = alloc.memorylocations[0].name
        if alloc.kind == "ExternalInput":
            if name == partition_name:
                continue
            in_names.append(name)
        elif alloc.kind == "ExternalOutput":
            out_names.append(name)
            shape = tuple(alloc.tensor_shape)
            dtype = mybir.dt.np(alloc.dtype)
            out_avals.append(jax.core.ShapedArray(shape, dtype))
            zero_outs.append(np.zeros(shape, dtype))
    n_params = len(in_names)
    all_names = in_names + out_names
    if partition_name is not None:
        all_names = all_names + [partition_name]

    from concourse.bass2jax import partition_id_tensor

    def _body(*args):
        operands = list(args)
        if partition_name is not None:
            operands.append(partition_id_tensor())
        outs = _bass_exec_p.bind(
            *operands,
            out_avals=tuple(out_avals),
            in_names=tuple(all_names),
            out_names=tuple(out_names),
            lowering_input_output_aliases=(),
            sim_require_finite=True,
            sim_require_nnan=True,
            nc=nc,
        )
        return tuple(outs)

    devices = jax.devices()[:NCORES]
    mesh = Mesh(np.asarray(devices), ("core",))
    nspec = (PartitionSpec("core"),) * (n_params + len(out_names))
    sharded = jax.jit(
        shard_map(_body, mesh=mesh, in_specs=nspec,
                  out_specs=(PartitionSpec("core"),) * len(out_names),
                  check_rep=False),
        keep_unused=True,
    )
    sharding = NamedSharding(mesh, PartitionSpec("core"))
    zeros_dev = [
        jax.device_put(
            np.zeros((NCORES * z.shape[0], *z.shape[1:]), z.dtype), sharding)
        for z in zero_outs
    ]
    _CACHE[key] = (sharded, sharding, in_names, out_names, zeros_dev)
    return _CACHE[key]


_CONST_NAMES = ("uw", "up", "v1")


def _device_inputs(in_names, in_maps, sharding):
    """device_put inputs; constant tensors are uploaded once and cached."""
    import jax
    ins = []
    for name in in_names:
        arr = np.concatenate([m[name] for m in in_maps], axis=0)
        if name in _CONST_NAMES:
            ck = ("dev", name)
            if ck not in _CACHE:
                _CACHE[ck] = jax.device_put(arr, sharding)
            ins.append(_CACHE[ck])
        else:
            ins.append(jax.device_put(arr, sharding))
    return ins


def _run(contour):
    sharded, sharding, in_names, out_names, zeros_dev = _get_executor()
    in_maps = _host_inputs(contour)
    ins = _device_inputs(in_names, in_maps, sharding)
    outs = sharded(*ins, *zeros_dev)
    res = []
    for c in range(NCORES):
        d = {}
        for i, name in enumerate(out_names):
            arr = np.asarray(outs[i])
            rows = arr.shape[0] // NCORES
            d[name] = arr[c * rows:(c + 1) * rows]
        res.append(d)
    return res


def benchmark(contour, iters=20, reps=1, skip=()):
    """Pipelined repeated execution; returns avg seconds/iteration."""
    import time
    import jax
    sharded, sharding, in_names, out_names, zeros_dev = _get_executor(
        reps, skip)
    in_maps = _host_inputs(np.asarray(contour, dtype=np.float32))
    ins = _device_inputs(in_names, in_maps, sharding)
    out = sharded(*ins, *zeros_dev)  # warm-up
    jax.block_until_ready(out)
    t0 = time.time()
    outs = [sharded(*ins, *zeros_dev) for _ in range(iters)]
    jax.block_until_ready(outs[-1])
    t1 = time.time()
    return (t1 - t0) / iters


def kernel(contour, *, _trace=False):
    contour = np.asarray(contour, dtype=np.float32)
    results = _run(contour)

    planes = []
    for p in range(NCORES):
        pr = results[p]["pr"]  # (128, 512) = [part, (bi, j)]
        planes.append(pr.reshape(128, 2, 256).transpose(1, 0, 2)
                      .reshape(SIZE, SIZE))
    prod2 = np.maximum(np.stack(planes), 0.0)
    dmap = np.sqrt(prod2)
    dmap = (dmap / dmap.max()).astype(np.float32)
    return dmap.reshape(2, 4, SIZE, SIZE)


# revision 10
# speedup vs baseline: 1.0904x; 1.0904x over previous
"""Contour -> distance map kernel for 8 Trainium2 NeuronCores.

Math. Per polygon p (one per core), the output map is
  dmap = wind(i,j) * dist(i,j) / global_max,
with wind the (tanh-smoothed, here exact-integer) winding number and dist
the distance from pixel (i/256, j/256) to the nearest contour VERTEX.

Both factors are piecewise structured along each pixel row i:

* Winding: cast a ray in +j. Edge crossings with the line u=i/256 happen at
  row-only-dependent positions v*_k(i); wind(i,j) = |sum of signed crossings
  right of j| = suffix-sum of a per-row signed histogram h(i,q) of the
  pixel-quantized crossing positions (exact: bins align with the j grid).

* Distance: dist^2(i,j) = min_k (j/256 - vy_k)^2 + a_k(i) is the lower
  envelope of 64 equal-curvature parabolas in v=j/256. Felzenszwalb's
  envelope (host, O(K) per row) makes it piecewise quadratic:
  dist^2 = C(i,j) - (j/256)*B2(i,j) with B2 = 2*vy_owner and
  C = vy_owner^2 + a_owner + (j/256)^2 piecewise-constant in j -> both are
  prefix sums of per-row jump histograms gB2, gC (the v^2 term folds into
  gC's bins as finite differences).

Device work per core: DMA 3 histograms (128c x 512) in, 12 accumulating
matmuls against constant triangular 0/1 matrices render the three maps
W (winding, signed), B2, D=C into PSUM, then 4 DVE tensor-tensor ops:
  W2 = W*W; m2 = B2*vj; d2 = D - m2; pr = W2*d2   [pr = (wind*dist)^2]
Host epilogue: sqrt + global max normalize (same role as the original
baseline's host epilogue).

Host prep is O(rows*K) per polygon (crossings + envelope), analogous to the
previous baseline's phi/W-matrix prep; all per-pixel work stays on device.
"""

import numpy as np

SIZE = 256
NCORES = 8
K = 64
DMA_SPLIT = 4
INF = 1e30

_CACHE = {}


def _build_program(reps=1, skip=()):
    import concourse.bacc as bacc
    import concourse.tile as tile
    from concourse import mybir

    f32 = mybir.dt.float32
    ALU = mybir.AluOpType
    AF = mybir.ActivationFunctionType

    nc = bacc.Bacc("TRN2", target_bir_lowering=False, debug=False,
                   num_devices=NCORES)

    hw_d = nc.dram_tensor("hw", [128, 512], f32, kind="ExternalInput")
    gb_d = nc.dram_tensor("gb", [128, 512], f32, kind="ExternalInput")
    gc_d = nc.dram_tensor("gc", [128, 512], f32, kind="ExternalInput")
    uw_d = nc.dram_tensor("uw", [128, 2, 256], f32, kind="ExternalInput")
    up_d = nc.dram_tensor("up", [128, 2, 256], f32, kind="ExternalInput")
    v1_d = nc.dram_tensor("v1", [128, 512], f32, kind="ExternalInput")
    pr_d = nc.dram_tensor("pr", [128, 512], f32, kind="ExternalOutput")

    with tile.TileContext(nc) as tc:
        with (
            tc.tile_pool(name="const", bufs=1) as const,
            tc.tile_pool(name="hbuf", bufs=2) as hbuf,
            tc.tile_pool(name="psum", bufs=2, space="PSUM") as psum_pool,
            tc.tile_pool(name="work", bufs=2) as work,
        ):
            uw_s = const.tile([128, 2, 256], f32)
            up_s = const.tile([128, 2, 256], f32)
            v1_s = const.tile([128, 512], f32)
            nc.sync.dma_start(uw_s[:], uw_d[:])
            nc.sync.dma_start(up_s[:], up_d[:])
            nc.sync.dma_start(v1_s[:], v1_d[:])

            import os
            nsplit = int(os.environ.get("ANT_DMA_SPLIT", "1"))
            spread = os.environ.get("ANT_QUEUES", "spread") == "spread"
            CW = 512 // nsplit
            q_in = {"hw": nc.scalar if spread else nc.sync,
                    "gb": nc.sync, "gc": nc.sync}
            q_out = nc.scalar if spread else nc.sync

            def body():
                hw_s = hbuf.tile([128, 512], f32, tag="hw")
                gb_s = hbuf.tile([128, 512], f32, tag="gb")
                gc_s = hbuf.tile([128, 512], f32, tag="gc")
                for nm, s, d in (("hw", hw_s, hw_d), ("gb", gb_s, gb_d),
                                 ("gc", gc_s, gc_d)):
                    for ch in range(nsplit):
                        sl = slice(ch * CW, (ch + 1) * CW)
                        q_in[nm].dma_start(s[:, sl], d[:, sl])

                W_t = psum_pool.tile([128, 512], f32, tag="W")
                B_t = psum_pool.tile([128, 512], f32, tag="B")
                D_t = psum_pool.tile([128, 512], f32, tag="D")
                # winding first so its DVE square overlaps the B/D matmuls
                for src, dst, rhs in ((hw_s, W_t, uw_s), (gb_s, B_t, up_s),
                                      (gc_s, D_t, up_s)):
                    for bi in range(2):
                        for bq in range(2):
                            col = (bq * 2 + bi) * 128
                            nc.tensor.matmul(
                                dst[:, 256 * bi:256 * bi + 256],
                                src[:, col:col + 128],
                                rhs[:, bq, :],
                                start=(bq == 0), stop=(bq == 1))

                W2 = work.tile([128, 512], f32, tag="W2")
                m2 = work.tile([128, 512], f32, tag="m2")
                d2 = work.tile([128, 512], f32, tag="d2")
                pr = work.tile([128, 512], f32, tag="pr")
                nc.scalar.activation(W2[:], W_t[:], AF.Square)
                nc.vector.tensor_tensor(m2[:], B_t[:], v1_s[:], op=ALU.mult)
                nc.vector.tensor_tensor(d2[:], D_t[:], m2[:], op=ALU.subtract)
                nc.vector.tensor_tensor(pr[:], W2[:], d2[:], op=ALU.mult)
                for ch in range(nsplit):
                    sl = slice(ch * CW, (ch + 1) * CW)
                    q_out.dma_start(pr_d[:, sl], pr[:, sl])

            if reps > 1:
                if os.environ.get("ANT_UNROLL") == "1":
                    for _ in range(reps):
                        body()
                else:
                    with tc.For_i(0, reps, 1,
                                  hint_engines=(mybir.EngineType.PE,
                                                mybir.EngineType.DVE)):
                        body()
            else:
                body()

    nc.compile()
    return nc


def _winding_hist(V):
    """Signed histogram of pixel-quantized ray crossings, (256 i, 256 q).
    wind(i, j) = |sum_{q-bin >= j} h[i, q]|."""
    A, B = V, np.roll(V, -1, axis=0)
    ui = (np.arange(SIZE) / SIZE)[:, None]
    Au, Av, Bu, Bv = A[:, 0][None], A[:, 1][None], B[:, 0][None], B[:, 1][None]
    du = Bu - Au
    active = ((Au <= ui) & (ui < Bu)) | ((Bu <= ui) & (ui < Au))
    with np.errstate(divide="ignore", invalid="ignore"):
        vstar = Av + (ui - Au) / du * (Bv - Av)
    s = np.where(du > 0, 1.0, -1.0)
    q = np.where(active, np.clip(np.ceil(SIZE * vstar), 0, SIZE), 0)
    q = q.astype(np.int64)
    sgn = np.where(active, s, 0.0)
    h = np.zeros((SIZE, SIZE))
    ii = np.broadcast_to(np.arange(SIZE)[:, None], q.shape)
    m = q > 0
    np.add.at(h, (ii[m], q[m] - 1), sgn[m])
    return h


def _envelope_hists(V):
    """Vectorized Felzenszwalb lower envelope over the 256 rows.
    Returns jump histograms gB2 (2*vy_owner) and gC (vy^2+a owner, with the
    v_j^2 term folded in); maps are prefix sums over q <= j."""
    R = SIZE
    order = np.argsort(V[:, 1], kind="stable")
    p = V[order, 1]
    u = (np.arange(R) / SIZE)[:, None]
    a = (u - V[order, 0][None]) ** 2
    c = a + p[None] ** 2
    stk_k = np.zeros((R, K), np.int64)
    stk_z = np.full((R, K), -INF)
    top = np.zeros(R, np.int64)
    rows = np.arange(R)

    def isect(q):
        kt = stk_k[rows, top]
        dp = p[q] - p[kt]
        ct = c[rows, kt]
        with np.errstate(divide="ignore", invalid="ignore"):
            z = (c[:, q] - ct) / (2 * dp)
        return np.where(dp == 0, np.where(c[:, q] >= ct, INF, -INF), z)

    for q in range(1, K):
        while True:
            z = isect(q)
            pop = (z <= stk_z[rows, top]) & (top > 0)
            if not pop.any():
                break
            top = top - pop
        z = isect(q)
        skip = z >= INF / 2
        repl = (~skip) & (z <= stk_z[rows, top])  # dominate-all: new base
        push = (~skip) & (~repl)
        stk_k[rows[repl], top[repl]] = q
        top = top + push
        stk_k[rows[push], top[push]] = q
        stk_z[rows[push], top[push]] = z[push]

    gB2 = np.zeros((R, SIZE))
    gC = np.zeros((R, SIZE))
    prevB = np.zeros(R)
    prevC = np.zeros(R)
    for s in range(int(top.max()) + 1):
        valid = top >= s
        ss = np.minimum(s, top)
        k_s = stk_k[rows, ss]
        z_s = stk_z[rows, ss]
        j0 = np.clip(np.ceil(SIZE * z_s), 0, 256).astype(np.int64)
        vB = 2 * p[k_s]
        vC = c[rows, k_s]
        m = valid & (j0 < 256)
        np.add.at(gB2, (rows[m], j0[m]), (vB - prevB)[m])
        np.add.at(gC, (rows[m], j0[m]), (vC - prevC)[m])
        prevB = np.where(m, vB, prevB)
        prevC = np.where(m, vC, prevC)
    vq = np.arange(SIZE) / SIZE
    gC = gC + np.concatenate([[0.0], np.diff(vq ** 2)])[None, :]
    return gB2, gC


def _lhsT(g):
    """(256 i, 256 q) map histogram -> matmul lhsT (128 c, 512) with column
    order (bq, bi, m): lhsT[c, bq*256+bi*128+m] = g[bi*128+m, bq*128+c]."""
    g4 = np.asarray(g, np.float32).reshape(2, 128, 2, 128)  # [bi, m, bq, c]
    return np.ascontiguousarray(g4.transpose(3, 2, 0, 1).reshape(128, 512))


def _consts():
    if "consts" in _CACHE:
        return _CACHE["consts"]
    b = np.arange(128)[:, None, None] + 128 * np.arange(2)[None, :, None]
    j = np.arange(256)[None, None, :]
    uw = (j <= b).astype(np.float32)                  # wind: suffix incl.
    up = (b <= j).astype(np.float32)                  # maps: prefix incl.
    v1 = np.ascontiguousarray(
        np.broadcast_to((np.arange(256) / SIZE).astype(np.float32),
                        (128, 2, 256)).reshape(128, 512))
    _CACHE["consts"] = {"uw": uw, "up": up, "v1": v1}
    return _CACHE["consts"]


def _host_inputs(contour):
    C = contour.reshape(NCORES, K, 2).astype(np.float64)
    consts = _consts()
    maps = []
    for pidx in range(NCORES):
        V = C[pidx]
        h = _winding_hist(V)
        gB2, gC = _envelope_hists(V)
        maps.append({"hw": _lhsT(h), "gb": _lhsT(gB2), "gc": _lhsT(gC),
                     **consts})
    return maps


def _get_executor(reps=1, skip=()):
    """Build (once) a reusable jitted SPMD executor over the 8 cores."""
    key = ("exec", reps, tuple(sorted(skip)))
    if key in _CACHE:
        return _CACHE[key]

    import jax
    from jax.sharding import Mesh, PartitionSpec, NamedSharding
    from jax.experimental.shard_map import shard_map
    import concourse.mybir as mybir
    from concourse.bass2jax import _bass_exec_p, install_neuronx_cc_hook

    install_neuronx_cc_hook()
    nckey = ("nc", reps, tuple(sorted(skip)))
    if nckey not in _CACHE:
        _CACHE[nckey] = _build_program(reps=reps, skip=skip)
    nc = _CACHE[nckey]
    partition_name = (nc.partition_id_tensor.name
                      if nc.partition_id_tensor else None)

    in_names, out_names, out_avals, zero_outs = [], [], [], []
    for alloc in nc.m.functions[0].allocations:
        if not isinstance(alloc, mybir.MemoryLocationSet):
            continue
        name = alloc.memorylocations[0].name
        if alloc.kind == "ExternalInput":
            if name == partition_name:
                continue
            in_names.append(name)
        elif alloc.kind == "ExternalOutput":
            out_names.append(name)
            shape = tuple(alloc.tensor_shape)
            dtype = mybir.dt.np(alloc.dtype)
            out_avals.append(jax.core.ShapedArray(shape, dtype))
            zero_outs.append(np.zeros(shape, dtype))
    n_params = len(in_names)
    all_names = in_names + out_names
    if partition_name is not None:
        all_names = all_names + [partition_name]

    from concourse.bass2jax import partition_id_tensor

    def _body(*args):
        operands = list(args)
        if partition_name is not None:
            operands.append(partition_id_tensor())
        outs = _bass_exec_p.bind(
            *operands,
            out_avals=tuple(out_avals),
            in_names=tuple(all_names),
            out_names=tuple(out_names),
            lowering_input_output_aliases=(),
            sim_require_finite=True,
            sim_require_nnan=True,
            nc=nc,
        )
        return tuple(outs)

    devices = jax.devices()[:NCORES]
    mesh = Mesh(np.asarray(devices), ("core",))
    nspec = (PartitionSpec("core"),) * (n_params + len(out_names))
    sharded = jax.jit(
        shard_map(_body, mesh=mesh, in_specs=nspec,
                  out_specs=(PartitionSpec("core"),) * len(out_names),
                  check_rep=False),
        keep_unused=True,
    )
    sharding = NamedSharding(mesh, PartitionSpec("core"))
    zeros_dev = [
        jax.device_put(
            np.zeros((NCORES * z.shape[0], *z.shape[1:]), z.dtype), sharding)
        for z in zero_outs
    ]
    _CACHE[key] = (sharded, sharding, in_names, out_names, zeros_dev)
    return _CACHE[key]


_CONST_NAMES = ("uw", "up", "v1")


def _device_inputs(in_names, in_maps, sharding):
    """device_put inputs; constant tensors are uploaded once and cached."""
    import jax
    ins = []
    for name in in_names:
        arr = np.concatenate([m[name] for m in in_maps], axis=0)
        if name in _CONST_NAMES:
            ck = ("dev", name)
            if ck not in _CACHE:
                _CACHE[ck] = jax.device_put(arr, sharding)
            ins.append(_CACHE[ck])
        else:
            ins.append(jax.device_put(arr, sharding))
    return ins


def _run(contour):
    sharded, sharding, in_names, out_names, zeros_dev = _get_executor()
    in_maps = _host_inputs(contour)
    ins = _device_inputs(in_names, in_maps, sharding)
    outs = sharded(*ins, *zeros_dev)
    res = []
    for c in range(NCORES):
        d = {}
        for i, name in enumerate(out_names):
            arr = np.asarray(outs[i])
            rows = arr.shape[0] // NCORES
            d[name] = arr[c * rows:(c + 1) * rows]
        res.append(d)
    return res


def benchmark(contour, iters=20, reps=1, skip=()):
    """Pipelined repeated execution; returns avg seconds/iteration."""
    import time
    import jax
    sharded, sharding, in_names, out_names, zeros_dev = _get_executor(
        reps, skip)
    in_maps = _host_inputs(np.asarray(contour, dtype=np.float32))
    ins = _device_inputs(in_names, in_maps, sharding)
    out = sharded(*ins, *zeros_dev)  # warm-up
    jax.block_until_ready(out)
    t0 = time.time()
    outs = [sharded(*ins, *zeros_dev) for _ in range(iters)]
    jax.block_until_ready(outs[-1])
    t1 = time.time()
    return (t1 - t0) / iters


def kernel(contour, *, _trace=False):
    contour = np.asarray(contour, dtype=np.float32)
    results = _run(contour)

    planes = []
    for p in range(NCORES):
        pr = results[p]["pr"]  # (128, 512) = [part, (bi, j)]
        planes.append(pr.reshape(128, 2, 256).transpose(1, 0, 2)
                      .reshape(SIZE, SIZE))
    prod2 = np.maximum(np.stack(planes), 0.0)
    dmap = np.sqrt(prod2)
    dmap = (dmap / dmap.max()).astype(np.float32)
    return dmap.reshape(2, 4, SIZE, SIZE)
